# revision 8
# baseline (speedup 1.0000x reference)
"""Bidirectional 2-layer GRU decoder on 8 TRN2 NeuronCores.

Strategy (pure SPMD, data-parallel over batch, no cross-core comms):
  - B=64 split 8 ways -> Bc=8 rows per core.
  - The 4 GRU cells (layer0 fwd, layer0 "bwd" (feature-flipped input),
    layer1 fwd, layer1 bwd) each own one PE column-group (32-partition
    quadrant): cell c -> PSUM partitions 32c..32c+7.
  - Per timestep, per cell, all matmuls accumulate in that cell's PSUM
    quadrant: bias (K=8 identity trick), gi = x_t @ WihT (input side,
    computed inline), gh = h_{t-1} @ WhhT.  The n-gate's gh part goes to a
    separate PSUM region (needed separately for r*gh_n).
  - Gates run as partition-parallel ACT/DVE ops spanning all quadrants.
  - h'^T (needed as next step's matmul stationary) is rebuilt with tiny
    col-tiled selector matmuls, no tiling-mode switches.
All matmul operands bf16 (fp32 PSUM accumulate), gates/state fp32.
"""

import os
import functools
import numpy as np
import ml_dtypes

import concourse.bass as bass
import concourse.mybir as mybir
import concourse.tile as tile
from concourse import bacc
from concourse.bass_utils import run_bass_kernel_spmd

try:
    import axon_prof
    axon_prof.install()
except Exception:
    pass

F32 = mybir.dt.float32
BF16 = mybir.dt.bfloat16
AF = mybir.ActivationFunctionType
OP = mybir.AluOpType

H = 512
D = 512
B = 64
S_FULL = 512
NCORES = 8
BC = B // NCORES          # batch rows per core = 8
CH = 32                   # x-chunk size in steps
PB = [0, 32, 64, 96]      # partition base per cell (0f, 0b, 1f, 1b)


def build_nc(S: int):
    ch = min(CH, S)
    nchunks = S // ch
    nc = bacc.Bacc("TRN2")

    # ---- DRAM I/O ----
    xtf = nc.dram_tensor("xtf", [nchunks, 128, ch * 32], BF16, kind="ExternalInput")
    xtb = nc.dram_tensor("xtb", [nchunks, 128, ch * 32], BF16, kind="ExternalInput")
    wih = nc.dram_tensor("wih", [4, 128, 4 * 1536], BF16, kind="ExternalInput")
    whh = nc.dram_tensor("whh", [4, 128, 4 * 1536], BF16, kind="ExternalInput")
    bias = nc.dram_tensor("bias", [128, 2048], BF16, kind="ExternalInput")
    eye = nc.dram_tensor("eye", [128, 32], BF16, kind="ExternalInput")
    sel = nc.dram_tensor("sel", [128, 32], F32, kind="ExternalInput")
    h0i = nc.dram_tensor("h0i", [2, 128, 512], F32, kind="ExternalInput")
    hti = nc.dram_tensor("hti", [2, 128, 64], BF16, kind="ExternalInput")
    out = nc.dram_tensor("out", [S, BC, 2 * H], F32, kind="ExternalOutput")

    with tile.TileContext(nc) as tc:
        with (
            tc.tile_pool(name="wpool", bufs=1) as wpool,
            tc.tile_pool(name="cpool", bufs=1) as cpool,
            tc.tile_pool(name="xpool", bufs=3) as xpool,
            tc.tile_pool(name="gpool", bufs=2) as gpool,
            tc.tile_pool(name="hpool", bufs=3) as hpool,
            tc.tile_pool(name="htpool", bufs=3) as htpool,
            tc.tile_pool(name="pA0", bufs=1, space="PSUM") as pA0,
            tc.tile_pool(name="pA1", bufs=1, space="PSUM") as pA1,
            tc.tile_pool(name="pN", bufs=1, space="PSUM") as pN,
            tc.tile_pool(name="pT", bufs=1, space="PSUM") as pT,
        ):
            # ---- constants / weights into SBUF ----
            wih_sb = [wpool.tile([128, 4 * 1536], BF16, tag=f"wih{c}", name=f"wih{c}") for c in range(4)]
            whh_sb = [wpool.tile([128, 4 * 1536], BF16, tag=f"whh{c}", name=f"whh{c}") for c in range(4)]
            for c in range(4):
                nc.sync.dma_start(wih_sb[c][:], wih[c])
                nc.sync.dma_start(whh_sb[c][:], whh[c])
            bias_sb = cpool.tile([128, 2048], BF16, tag="bias")
            eye_sb = cpool.tile([128, 32], BF16, tag="eye")
            sel_sb = cpool.tile([128, 32], F32, tag="sel")
            nc.sync.dma_start(bias_sb[:], bias[:])
            nc.sync.dma_start(eye_sb[:], eye[:])
            nc.sync.dma_start(sel_sb[:], sel[:])

            # persistent PSUM tiles (junk rows memset once, never rewritten)
            psA = [pA0.tile([128, 1536], F32, tag="psA0", name="psA0"),
                   pA1.tile([128, 1536], F32, tag="psA1", name="psA1")]
            psN = pN.tile([128, 512], F32, tag="psN")
            psT = pT.tile([128, 128], F32, tag="psT")
            nc.vector.memset(psA[0][:], 0.0)
            nc.vector.memset(psA[1][:], 0.0)
            nc.vector.memset(psN[:], 0.0)
            nc.vector.memset(psT[:], 0.0)

            # initial states
            h_prev = [hpool.tile([128, 512], F32, tag="hp0", name="hp0i"),
                      hpool.tile([128, 512], F32, tag="hp1", name="hp1i")]
            ht_prev = [htpool.tile([128, 64], BF16, tag="ht0", name="ht0i"),
                       htpool.tile([128, 64], BF16, tag="ht1", name="ht1i")]
            for L in range(2):
                nc.sync.dma_start(h_prev[L][:], h0i[L])
                nc.sync.dma_start(ht_prev[L][:], hti[L])

            xs_tiles = {}

            def load_chunk(ci):
                tf = xpool.tile([128, ch * 32], BF16, tag="xsf")
                tb = xpool.tile([128, ch * 32], BF16, tag="xsb")
                nc.sync.dma_start(tf[:], xtf[ci])
                nc.sync.dma_start(tb[:], xtb[ci])
                xs_tiles[ci] = (tf, tb)

            load_chunk(0)

            SP = slice(0, 104)  # gate ops span (junk rows harmless: psum zeros)

            def mm_slots(L, cx, t):
                """28 (out_ap-fn, lhsT, rhs, start, stop) slots for one cell."""
                c = 2 * L + cx
                pb = PB[c]
                ci, dt = t // ch, t % ch
                A = psA[L]
                o = A[pb:pb + 8, :]
                oN = psN[pb:pb + 8, :]
                eyec = eye_sb[:, 8 * c:8 * c + 8]
                slots = []
                for lo in (0, 512, 1024):
                    slots.append((o[:, lo:lo + 512], eyec,
                                  bias_sb[:, lo:lo + 512], True, False))
                for k in range(4):
                    if L == 0:
                        st = xs_tiles[ci][cx][:, (dt * 4 + k) * 8:(dt * 4 + k) * 8 + 8]
                    else:
                        st = ht_prev[0][:, k * 16 + 8 * cx:k * 16 + 8 * cx + 8]
                    for lo in (0, 512, 1024):
                        slots.append((o[:, lo:lo + 512], st,
                                      wih_sb[c][:, k * 1536 + lo:k * 1536 + lo + 512],
                                      False, (lo == 1024 and k == 3)))
                for k in range(4):
                    st = ht_prev[L][:, k * 16 + 8 * cx:k * 16 + 8 * cx + 8]
                    for lo in (0, 512):
                        slots.append((o[:, lo:lo + 512], st,
                                      whh_sb[c][:, k * 1536 + lo:k * 1536 + lo + 512],
                                      False, (k == 3)))
                slots.append((oN, eyec, bias_sb[:, 1536:2048], True, False))
                for k in range(4):
                    st = ht_prev[L][:, k * 16 + 8 * cx:k * 16 + 8 * cx + 8]
                    slots.append((oN, st,
                                  whh_sb[c][:, k * 1536 + 1024:k * 1536 + 1536],
                                  False, (k == 3)))
                return pb, slots

            def gates(L, t):
                A = psA[L]
                new_ht = htpool.tile([128, 64], BF16, tag=f"ht{L}", name=f"ht{L}_{t}")
                r_t = gpool.tile([128, 512], F32, tag=f"r{L}", name=f"r{L}_{t}")
                z_t = gpool.tile([128, 512], F32, tag=f"z{L}", name=f"z{L}_{t}")
                m1 = gpool.tile([128, 512], F32, tag=f"m1{L}", name=f"m1{L}_{t}")
                a3 = gpool.tile([128, 512], F32, tag=f"a3{L}", name=f"a3{L}_{t}")
                n_t = gpool.tile([128, 512], F32, tag=f"n{L}", name=f"n{L}_{t}")
                s1 = gpool.tile([128, 512], F32, tag=f"s1{L}", name=f"s1{L}_{t}")
                m2 = gpool.tile([128, 512], F32, tag=f"m2{L}", name=f"m2{L}_{t}")
                hp = hpool.tile([128, 512], F32, tag=f"hp{L}", name=f"hp{L}_{t}")
                nc.scalar.activation(r_t[SP], A[SP, 0:512], AF.Sigmoid)
                nc.vector.tensor_tensor(m1[SP], r_t[SP], psN[SP, :], OP.mult)
                nc.vector.tensor_tensor(a3[SP], m1[SP], A[SP, 1024:1536], OP.add)
                nc.scalar.activation(z_t[SP], A[SP, 512:1024], AF.Sigmoid)
                nc.scalar.activation(n_t[SP], a3[SP], AF.Tanh)
                nc.gpsimd.tensor_sub(s1[SP], h_prev[L][SP], n_t[SP])
                nc.gpsimd.tensor_mul(m2[SP], z_t[SP], s1[SP])
                nc.vector.tensor_tensor(hp[SP], n_t[SP], m2[SP], OP.add)
                # rebuild h'^T via selector matmuls
                for k in range(4):
                    for g in range(4):
                        nc.tensor.matmul(
                            psT[32 * g:32 * g + 32, 64 * L + 16 * k:64 * L + 16 * k + 16],
                            hp[0:104, 128 * k + 32 * g:128 * k + 32 * g + 32],
                            sel_sb[0:104, 16 * L:16 * L + 16],
                            start=True, stop=True, tile_position=(0, 32 * g),
                            skip_group_check=True)
                nc.vector.tensor_copy(new_ht[:], psT[:, 64 * L:64 * L + 64])
                h_prev[L] = hp
                ht_prev[L] = new_ht
                return hp

            # software pipeline: tick tau runs L0 @ step tau and L1 @ step tau-1,
            # with the 4 cells' matmuls interleaved so all 4 PE column groups
            # stream concurrently.
            for tau in range(S + 1):
                ci = (tau // ch) if tau < S else None
                if tau < S and tau % ch == 0 and ci + 1 < nchunks:
                    load_chunk(ci + 1)
                work = []
                if tau < S:
                    work += [mm_slots(0, 0, tau), mm_slots(0, 1, tau)]
                if tau >= 1:
                    work += [mm_slots(1, 0, tau - 1), mm_slots(1, 1, tau - 1)]
                for i in range(28):
                    for pb, slots in work:
                        o_ap, lhsT, rhs, st_, sp_ = slots[i]
                        nc.tensor.matmul(o_ap, lhsT, rhs, start=st_, stop=sp_,
                                         tile_position=(0, pb),
                                         skip_group_check=True)
                if tau < S:
                    gates(0, tau)
                if tau >= 1:
                    hp1 = gates(1, tau - 1)
                    nc.sync.dma_start(out[tau - 1, :, 0:512], hp1[64:72, :])
                    nc.sync.dma_start(out[tau - 1, :, 512:1024], hp1[96:104, :])

    nc.compile()
    return nc


# ---------------- host-side data prep ----------------

def _gru_splits(Wih, Whh, bih, bhh):
    """Return (wih_sw[128,4*1536], whh_sw[128,4*1536], biasA[1536], biasB[512])."""
    wihT = np.ascontiguousarray(Wih.T)            # [512, 1536]
    whhT = np.ascontiguousarray(Whh.T)
    wih_sw = wihT.reshape(4, 128, 1536).transpose(1, 0, 2).reshape(128, 4 * 1536)
    whh_sw = whhT.reshape(4, 128, 1536).transpose(1, 0, 2).reshape(128, 4 * 1536)
    biasA = np.concatenate([bih[0:512] + bhh[0:512],
                            bih[512:1024] + bhh[512:1024],
                            bih[1024:1536]])
    biasB = bhh[1024:1536]
    return wih_sw, whh_sw, biasA, biasB


@functools.lru_cache(maxsize=2)
def _get_nc(S):
    return build_nc(S)


def _prep_inputs(input, encoder_h, params, S):
    """params: list of 4 (Wih, Whh, bih, bhh) for cells [0f, 0b, 1f, 1b].
    Returns (shared_map, per_core_maps)."""
    ch = min(CH, S)
    nchunks = S // ch
    bf = ml_dtypes.bfloat16

    wih_all = np.zeros((4, 128, 4 * 1536), np.float32)
    whh_all = np.zeros((4, 128, 4 * 1536), np.float32)
    bias_all = np.zeros((128, 2048), np.float32)
    for c in range(4):
        wih_sw, whh_sw, biasA, biasB = _gru_splits(*params[c])
        wih_all[c] = wih_sw
        whh_all[c] = whh_sw
        bias_all[8 * c:8 * c + 8, 0:1536] = biasA[None, :]
        bias_all[8 * c:8 * c + 8, 1536:2048] = biasB[None, :]
    eye = np.zeros((128, 32), np.float32)
    for c in range(4):
        for j in range(8):
            eye[8 * c + j, 8 * c + j] = 1.0
    sel = np.zeros((128, 32), np.float32)
    for L in range(2):
        for cx in range(2):
            for j in range(8):
                sel[64 * L + 32 * cx + j, 16 * L + 8 * cx + j] = 1.0

    shared = {
        "wih": wih_all.astype(bf),
        "whh": whh_all.astype(bf),
        "bias": bias_all.astype(bf),
        "eye": eye.astype(bf),
        "sel": sel,
    }

    per_core = []
    for r in range(NCORES):
        bs = slice(r * BC, (r + 1) * BC)
        xc = input[bs, :S]                       # [8, S, 512]
        # xT[t, i, b] = x[b, t, i]; swizzled to [nchunks, 128, CH*4*8]
        def swz(xarr):
            xt = xarr.transpose(1, 2, 0).reshape(nchunks, ch, 4, 128, BC)
            return np.ascontiguousarray(xt.transpose(0, 3, 1, 2, 4)
                                        ).reshape(nchunks, 128, ch * 32).astype(bf)
        xtf = swz(xc)
        xtb = swz(xc[:, :, ::-1])
        hf = encoder_h[bs, 0:512]                # [8, 512]
        hb = encoder_h[bs, 512:1024]
        h0i = np.zeros((2, 128, 512), np.float32)
        hti = np.zeros((2, 128, 64), np.float32)
        for L in range(2):
            h0i[L, 0 + 64 * L:8 + 64 * L] = hf
            h0i[L, 32 + 64 * L:40 + 64 * L] = hb
            for k in range(4):
                hti[L, :, 16 * k + 0:16 * k + 8] = hf.T[128 * k:128 * k + 128]
                hti[L, :, 16 * k + 8:16 * k + 16] = hb.T[128 * k:128 * k + 128]
        per_core.append(dict(shared, xtf=xtf, xtb=xtb,
                             h0i=h0i, hti=hti.astype(bf)))
    return per_core


def _run(inputs, S=S_FULL, trace=False):
    p0f = (inputs["gru0_f_Wih"], inputs["gru0_f_Whh"], inputs["gru0_f_bih"], inputs["gru0_f_bhh"])
    p0b = (inputs["gru0_b_Wih"], inputs["gru0_b_Whh"], inputs["gru0_b_bih"], inputs["gru0_b_bhh"])
    p1f = (inputs["gru1_f_Wih"], inputs["gru1_f_Whh"], inputs["gru1_f_bih"], inputs["gru1_f_bhh"])
    p1b = (inputs["gru1_b_Wih"], inputs["gru1_b_Whh"], inputs["gru1_b_bih"], inputs["gru1_b_bhh"])
    params = [tuple(np.asarray(a, np.float32) for a in p) for p in (p0f, p0b, p1f, p1b)]
    x = np.asarray(inputs["input"], np.float32)
    eh = np.asarray(inputs["encoder_h"], np.float32)

    nc = _get_nc(S)
    in_maps = _prep_inputs(x, eh, params, S)
    res = run_bass_kernel_spmd(nc, in_maps, core_ids=list(range(NCORES)), trace=trace)
    outs = [res.results[r]["out"] for r in range(NCORES)]   # [S, 8, 1024] each
    full = np.concatenate([o.transpose(1, 0, 2) for o in outs], axis=0)  # [B, S, 1024]
    return full, res


def kernel(**inputs):
    full, _ = _run(inputs)
    return full, np.ascontiguousarray(full[:, -1, :])


# revision 17
# speedup vs baseline: 1.0761x; 1.0761x over previous
"""Bidirectional 2-layer GRU decoder on 8 TRN2 NeuronCores.

Strategy (pure SPMD, data-parallel over batch, no cross-core comms):
  - B=64 split 8 ways -> Bc=8 rows per core.
  - The 4 GRU cells (layer0 fwd, layer0 "bwd" (feature-flipped input),
    layer1 fwd, layer1 bwd) each own one PE column-group (32-partition
    quadrant): cell c -> PSUM partitions 32c..32c+7.
  - Input-side matmuls (gi = x@WihT + biasA) are precomputed in large
    batched matmuls: layer0 upfront into DRAM, layer1 in 16-step chunks
    from the h0^T history ring (so layer1 lags layer0 by LAG steps).
    Per step they are injected into each cell's PSUM quadrant with a
    K=128 identity matmul (a partition-aligning accumulate), then the
    recurrent gh matmuls accumulate on top.
  - Gates run as partition-parallel ACT/DVE/GpSimd ops spanning all
    quadrants; h'^T for the next step's stationary is rebuilt with tiny
    col-tiled selector matmuls into the spent r-gate PSUM bank.
  - Emission interleaves the 4 active cells (L0 @ step tau, L1 @ step
    tau-LAG) across the 4 PE column groups for concurrent streaming.
All matmul operands bf16 (fp32 PSUM accumulate), recurrent state fp32.
"""

import functools
import numpy as np
import ml_dtypes

import concourse.bass as bass
import concourse.mybir as mybir
import concourse.tile as tile
from concourse import bacc
from concourse.bass_utils import run_bass_kernel_spmd

try:
    import axon_prof
    axon_prof.install()
except Exception:
    pass

F32 = mybir.dt.float32
BF16 = mybir.dt.bfloat16
AF = mybir.ActivationFunctionType
OP = mybir.AluOpType

H = 512
B = 64
S_FULL = 512
NCORES = 8
BC = B // NCORES          # batch rows per core = 8
CH = 32                   # x-chunk size (phase-1 streaming)
CG = 16                   # gi1 chunk size in steps
LAG = 18                  # layer-1 step lag behind layer-0
PB = [0, 32, 64, 96]      # partition base per cell (0f, 0b, 1f, 1b)


def build_nc(S: int):
    ch = min(CH, S)
    nchunks = S // ch
    cg = min(CG, S)
    lag = LAG if S > CG else (cg + 2)
    nc = bacc.Bacc("TRN2")

    # ---- DRAM I/O ----
    ngrp = (S + 15) // 16
    xtf = nc.dram_tensor("xtf", [ngrp, 128, 512], BF16, kind="ExternalInput")
    xtb = nc.dram_tensor("xtb", [ngrp, 128, 512], BF16, kind="ExternalInput")
    wih = nc.dram_tensor("wih", [4, 128, 4 * 1536], BF16, kind="ExternalInput")
    whh = nc.dram_tensor("whh", [4, 128, 4 * 1536], BF16, kind="ExternalInput")
    biasn = nc.dram_tensor("biasn", [128, 512], BF16, kind="ExternalInput")
    biasa = nc.dram_tensor("biasa", [128, 4 * 1536], BF16, kind="ExternalInput")
    eye = nc.dram_tensor("eye", [128, 128], BF16, kind="ExternalInput")
    sel = nc.dram_tensor("sel", [128, 32], F32, kind="ExternalInput")
    h0i = nc.dram_tensor("h0i", [2, 128, 512], F32, kind="ExternalInput")
    hti = nc.dram_tensor("hti", [2, 128, 64], BF16, kind="ExternalInput")
    out = nc.dram_tensor("out", [S, BC, 2 * H], F32, kind="ExternalOutput")
    gi0d = nc.dram_tensor("gi0d", [(S + 15) // 16, 2, 128, 1536], BF16)  # internal

    with tile.TileContext(nc) as tc:
        with (
            tc.tile_pool(name="wpool", bufs=1) as wpool,
            tc.tile_pool(name="cpool", bufs=1) as cpool,
            tc.tile_pool(name="xpool", bufs=3) as xpool,
            tc.tile_pool(name="stpool", bufs=3) as stpool,
            tc.tile_pool(name="gpool", bufs=2) as gpool,
            tc.tile_pool(name="hpool", bufs=3) as hpool,
            tc.tile_pool(name="htpool", bufs=3) as htpool,
            tc.tile_pool(name="gi1pool", bufs=2) as gi1pool,
            tc.tile_pool(name="pA0", bufs=1, space="PSUM") as pA0,
            tc.tile_pool(name="pA1", bufs=1, space="PSUM") as pA1,
            tc.tile_pool(name="pN0", bufs=1, space="PSUM") as pN0,
            tc.tile_pool(name="pN1", bufs=1, space="PSUM") as pN1,
        ):
            # ---- constants / weights ----
            wih_sb = [wpool.tile([128, 4 * 1536], BF16, tag=f"wih{c}", name=f"wih{c}") for c in range(4)]
            whh_sb = [wpool.tile([128, 4 * 1536], BF16, tag=f"whh{c}", name=f"whh{c}") for c in range(4)]
            for c in range(4):
                nc.sync.dma_start(wih_sb[c][:], wih[c])
                nc.sync.dma_start(whh_sb[c][:], whh[c])
            biasn_sb = cpool.tile([128, 512], BF16, tag="biasn")
            biasa_sb = cpool.tile([128, 4 * 1536], BF16, tag="biasa")
            eye_sb = cpool.tile([128, 128], BF16, tag="eye")
            sel_sb = cpool.tile([128, 32], F32, tag="sel")
            nc.sync.dma_start(biasn_sb[:], biasn[:])
            nc.sync.dma_start(biasa_sb[:], biasa[:])
            nc.sync.dma_start(eye_sb[:], eye[:])
            nc.sync.dma_start(sel_sb[:], sel[:])

            psA = [pA0.tile([128, 1536], F32, tag="psA0", name="psA0"),
                   pA1.tile([128, 1536], F32, tag="psA1", name="psA1")]
            psN = [pN0.tile([128, 512], F32, tag="psN0", name="psN0"),
                   pN1.tile([128, 512], F32, tag="psN1", name="psN1")]

            # ---- phase 1: gi0 = x @ Wih0^T (+biasA) for both L0 cells ----
            for gi_ in range(ngrp):
                xf = xpool.tile([128, 512], BF16, tag="xsf", name=f"xf{gi_}")
                xb = xpool.tile([128, 512], BF16, tag="xsb", name=f"xb{gi_}")
                nc.sync.dma_start(xf[:], xtf[gi_])
                nc.sync.dma_start(xb[:], xtb[gi_])
                for cx, xs in ((0, xf), (1, xb)):
                    stg = stpool.tile([128, 1536], BF16, tag="stg",
                                      name=f"st{gi_}_{cx}")
                    for lo in (0, 512, 1024):
                        for k in range(4):
                            nc.tensor.matmul(
                                psA[0][0:128, 0:512],
                                xs[:, k * 128:k * 128 + 128],
                                wih_sb[cx][:, k * 1536 + lo:k * 1536 + lo + 512],
                                start=(k == 0), stop=(k == 3),
                                tile_position=(0, 0), skip_group_check=True)
                        nc.vector.tensor_tensor(
                            stg[:, lo:lo + 512],
                            psA[0][0:128, 0:512],
                            biasa_sb[:, cx * 1536 + lo:cx * 1536 + lo + 512],
                            OP.add)
                    nc.sync.dma_start(out=gi0d[gi_, cx, :, :], in_=stg[:, :])

            # zero psums (junk rows stay zero through the whole main loop)
            nc.vector.memset(psA[0][:], 0.0)
            nc.vector.memset(psA[1][:], 0.0)
            nc.vector.memset(psN[0][:], 0.0)
            nc.vector.memset(psN[1][:], 0.0)

            # ---- states ----
            h_prev = [hpool.tile([128, 512], F32, tag="hp0", name="hp0i"),
                      hpool.tile([128, 512], F32, tag="hp1", name="hp1i")]
            ht1_prev = htpool.tile([128, 64], BF16, tag="ht1", name="ht1i")
            ring = cpool.tile([128, 2048], BF16, tag="ring")   # h0^T history (2 halves)
            ht0_init = cpool.tile([128, 64], BF16, tag="ht0i")
            for L in range(2):
                nc.sync.dma_start(h_prev[L][:], h0i[L])
            nc.sync.dma_start(ht0_init[:], hti[0])
            nc.sync.dma_start(ht1_prev[:], hti[1])

            gi0_sb = {}   # chunk -> (tile_f, tile_b)
            gi1_sb = {}   # chunk -> (tile_f, tile_b)

            def gi0_chunk_load(C):
                tiles = []
                for cx in range(2):
                    g = gi1pool.tile([128, 1536], BF16, tag=f"gi0_{cx}",
                                     name=f"gi0c_{cx}_{C}")
                    nc.sync.dma_start(g[:], gi0d[C, cx])
                    tiles.append(g)
                gi0_sb[C] = tiles

            gi0_chunk_load(0)
            ring_v = ring.rearrange("p (hf k c d b) -> p hf k c d b",
                                    hf=2, k=4, c=2, d=16, b=8)

            SP = slice(0, 104)

            def ht0_slice(t, k, cx):
                if t < 0:
                    return ht0_init[:, k * 16 + 8 * cx:k * 16 + 8 * cx + 8]
                hf = (t // 16) % 2
                base = hf * 1024 + k * 256 + cx * 128 + (t % 16) * 8
                return ring[:, base:base + 8]

            def gi1_chunk_mm(C):
                """Batched gi1 for steps [C*cg, C*cg+cg) from the ring."""
                tiles = []
                ns = min(cg, S - C * cg)
                s0 = (C % 2) * 1024
                for cx in range(2):
                    c = 2 + cx
                    g = gi1pool.tile([128, 1536], BF16, tag=f"gi1_{cx}",
                                     name=f"gi1_{cx}_{C}")
                    if ns < cg or cg < 16:
                        nc.vector.memset(g[:, :], 0.0)
                    ps = psN[cx]
                    for lo in (0, 512, 1024):
                        for k in range(4):
                            lhsT = ring[:, s0 + k * 256 + cx * 128:
                                        s0 + k * 256 + cx * 128 + ns * 8]
                            nc.tensor.matmul(
                                ps[0:ns * 8, 0:512], lhsT,
                                wih_sb[c][:, k * 1536 + lo:k * 1536 + lo + 512],
                                start=(k == 0), stop=(k == 3),
                                tile_position=(0, 0), skip_group_check=True)
                        nc.vector.tensor_tensor(
                            g[0:ns * 8, lo:lo + 512], ps[0:ns * 8, 0:512],
                            biasa_sb[0:ns * 8, c * 1536 + lo:c * 1536 + lo + 512],
                            OP.add)
                    tiles.append(g)
                gi1_sb[C] = tiles

            def mm_slots(L, cx, t):
                """16 (out, lhsT, rhs, start, stop) slots for one cell-step."""
                c = 2 * L + cx
                pb = PB[c]
                A = psA[L]
                o = A[pb:pb + 8, :]
                oN = psN[L][pb:pb + 8, :]
                gsrc = (gi0_sb if L == 0 else gi1_sb)[t // cg][cx]
                esl = eye_sb[:, (t % cg) * 8:(t % cg) * 8 + 8]
                gl = [gsrc[:, lo:lo + 512] for lo in (0, 512, 1024)]
                hts = (lambda k: ht0_slice(t - 1, k, cx)) if L == 0 else \
                      (lambda k: ht1_prev[:, k * 16 + 8 * cx:k * 16 + 8 * cx + 8])
                slots = []
                # r bank: gi preload + gh-r
                slots.append((o[:, 0:512], esl, gl[0], True, False))
                for k in range(4):
                    slots.append((o[:, 0:512], hts(k),
                                  whh_sb[c][:, k * 1536:k * 1536 + 512],
                                  False, (k == 3)))
                # psN bank: bhn bias + gh-n
                slots.append((oN, eye_sb[:, 8 * c:8 * c + 8], biasn_sb[:], True, False))
                for k in range(4):
                    slots.append((oN, hts(k),
                                  whh_sb[c][:, k * 1536 + 1024:k * 1536 + 1536],
                                  False, (k == 3)))
                # n-gi bank: gi preload only
                slots.append((o[:, 1024:1536], esl, gl[2], True, True))
                # z bank: gi preload + gh-z
                slots.append((o[:, 512:1024], esl, gl[1], True, False))
                for k in range(4):
                    slots.append((o[:, 512:1024], hts(k),
                                  whh_sb[c][:, k * 1536 + 512:k * 1536 + 1024],
                                  False, (k == 3)))
                return pb, slots

            def gates(L, t):
                A = psA[L]
                pN_ = psN[L]
                sfx = f"{L}_{t}"
                r_t = gpool.tile([128, 512], BF16, tag=f"r{L}", name=f"r{sfx}")
                z_t = gpool.tile([128, 512], BF16, tag=f"z{L}", name=f"z{sfx}")
                oz = gpool.tile([128, 512], BF16, tag=f"oz{L}", name=f"oz{sfx}")
                zh = gpool.tile([128, 512], F32, tag=f"zh{L}", name=f"zh{sfx}")
                m1 = gpool.tile([128, 512], BF16, tag=f"m1{L}", name=f"m1{sfx}")
                a3 = gpool.tile([128, 512], BF16, tag=f"a3{L}", name=f"a3{sfx}")
                n_t = gpool.tile([128, 512], BF16, tag=f"n{L}", name=f"n{sfx}")
                m2 = gpool.tile([128, 512], BF16, tag=f"m2{L}", name=f"m2{sfx}")
                hp = hpool.tile([128, 512], F32, tag=f"hp{L}", name=f"hp{sfx}")
                nc.scalar.activation(r_t[SP], A[SP, 0:512], AF.Sigmoid)
                nc.vector.tensor_tensor(m1[SP], r_t[SP], pN_[SP, :], OP.mult)
                nc.vector.tensor_tensor(a3[SP], m1[SP], A[SP, 1024:1536], OP.add)
                nc.scalar.activation(z_t[SP], A[SP, 512:1024], AF.Sigmoid)
                # oz = 1 - z ; zh = z * h   (off critical path)
                nc.vector.tensor_scalar(oz[SP], z_t[SP], -1.0, 1.0,
                                        OP.mult, OP.add)
                nc.gpsimd.tensor_tensor(zh[SP], z_t[SP], h_prev[L][SP], OP.mult)
                nc.scalar.activation(n_t[SP], a3[SP], AF.Tanh)
                nc.vector.tensor_tensor(m2[SP], oz[SP], n_t[SP], OP.mult)
                nc.vector.tensor_tensor(hp[SP], m2[SP], zh[SP], OP.add)
                # h'^T selector matmuls into the spent r bank (cols 0:64)
                for k in range(4):
                    for g in range(4):
                        nc.tensor.matmul(
                            A[32 * g:32 * g + 32, 16 * k:16 * k + 16],
                            hp[0:104, 128 * k + 32 * g:128 * k + 32 * g + 32],
                            sel_sb[0:104, 16 * L:16 * L + 16],
                            start=True, stop=True, tile_position=(0, 32 * g),
                            skip_group_check=True)
                h_prev[L] = hp
                if L == 0:
                    hf = (t // 16) % 2
                    nc.vector.tensor_copy(
                        ring_v[:, hf, :, :, t % 16, :],
                        A[:, 0:64].rearrange("p (k c b) -> p k c b", k=4, c=2))
                    return hp, None
                nht = htpool.tile([128, 64], BF16, tag="ht1", name=f"ht1_{t}")
                nc.vector.tensor_copy(nht[:], A[:, 0:64])
                return hp, nht

            # ---- main loop ----
            for tau in range(S + lag):
                if tau % cg == 0 and tau >= cg:
                    C = (tau - cg) // cg
                    if C * cg < S:
                        gi1_chunk_mm(C)
                if tau % cg == 0 and tau + cg < S:
                    gi0_chunk_load((tau + cg) // cg)

                work = []
                if tau < S:
                    work.append(mm_slots(0, 0, tau))
                    work.append(mm_slots(0, 1, tau))
                t1 = tau - lag
                if 0 <= t1 < S:
                    work.append(mm_slots(1, 0, t1))
                    work.append(mm_slots(1, 1, t1))
                for i in range(16):
                    for pb, slots in work:
                        o_ap, lhsT, rhs, st_, sp_ = slots[i]
                        nc.tensor.matmul(o_ap, lhsT, rhs, start=st_, stop=sp_,
                                         tile_position=(0, pb),
                                         skip_group_check=True)
                if tau < S:
                    gates(0, tau)
                if 0 <= t1 < S:
                    hp1, nht = gates(1, t1)
                    ht1_prev = nht
                    nc.sync.dma_start(out[t1, :, 0:512], hp1[64:72, :])
                    nc.sync.dma_start(out[t1, :, 512:1024], hp1[96:104, :])

    nc.compile()
    return nc


# ---------------- host-side data prep ----------------

def _gru_splits(Wih, Whh, bih, bhh):
    wihT = np.ascontiguousarray(Wih.T)
    whhT = np.ascontiguousarray(Whh.T)
    wih_sw = wihT.reshape(4, 128, 1536).transpose(1, 0, 2).reshape(128, 4 * 1536)
    whh_sw = whhT.reshape(4, 128, 1536).transpose(1, 0, 2).reshape(128, 4 * 1536)
    biasA = np.concatenate([bih[0:512] + bhh[0:512],
                            bih[512:1024] + bhh[512:1024],
                            bih[1024:1536]])
    biasB = bhh[1024:1536]
    return wih_sw, whh_sw, biasA, biasB


@functools.lru_cache(maxsize=2)
def _get_nc(S):
    return build_nc(S)


def _prep_inputs(input, encoder_h, params, S):
    ngrp = (S + 15) // 16
    bf = ml_dtypes.bfloat16

    wih_all = np.zeros((4, 128, 4 * 1536), np.float32)
    whh_all = np.zeros((4, 128, 4 * 1536), np.float32)
    biasn = np.zeros((128, 512), np.float32)
    biasa = np.zeros((128, 4 * 1536), np.float32)
    for c in range(4):
        wih_sw, whh_sw, biasA, biasB = _gru_splits(*params[c])
        wih_all[c] = wih_sw
        whh_all[c] = whh_sw
        biasa[:, c * 1536:(c + 1) * 1536] = biasA[None, :]
        biasn[8 * c:8 * c + 8, :] = biasB[None, :]
    eye = np.eye(128, dtype=np.float32)
    sel = np.zeros((128, 32), np.float32)
    for L in range(2):
        for cx in range(2):
            for j in range(8):
                sel[64 * L + 32 * cx + j, 16 * L + 8 * cx + j] = 1.0

    shared = {
        "wih": wih_all.astype(bf),
        "whh": whh_all.astype(bf),
        "biasn": biasn.astype(bf),
        "biasa": biasa.astype(bf),
        "eye": eye.astype(bf),
        "sel": sel,
    }

    per_core = []
    for r in range(NCORES):
        bs = slice(r * BC, (r + 1) * BC)
        xc = input[bs, :S]

        def swz(xarr):
            xp = np.zeros((BC, ngrp * 16, 512), np.float32)
            xp[:, :S] = xarr
            # [b, g, dt, k, p] -> [g, p, k, dt, b]
            xt = xp.reshape(BC, ngrp, 16, 4, 128).transpose(1, 4, 3, 2, 0)
            return np.ascontiguousarray(xt).reshape(ngrp, 128, 512).astype(bf)
        xtf = swz(xc)
        xtb = swz(xc[:, :, ::-1])
        hf = encoder_h[bs, 0:512]
        hb = encoder_h[bs, 512:1024]
        h0i = np.zeros((2, 128, 512), np.float32)
        hti = np.zeros((2, 128, 64), np.float32)
        for L in range(2):
            h0i[L, 0 + 64 * L:8 + 64 * L] = hf
            h0i[L, 32 + 64 * L:40 + 64 * L] = hb
            for k in range(4):
                hti[L, :, 16 * k + 0:16 * k + 8] = hf.T[128 * k:128 * k + 128]
                hti[L, :, 16 * k + 8:16 * k + 16] = hb.T[128 * k:128 * k + 128]
        per_core.append(dict(shared, xtf=xtf, xtb=xtb,
                             h0i=h0i, hti=hti.astype(bf)))
    return per_core


def _run(inputs, S=S_FULL, trace=False):
    p0f = (inputs["gru0_f_Wih"], inputs["gru0_f_Whh"], inputs["gru0_f_bih"], inputs["gru0_f_bhh"])
    p0b = (inputs["gru0_b_Wih"], inputs["gru0_b_Whh"], inputs["gru0_b_bih"], inputs["gru0_b_bhh"])
    p1f = (inputs["gru1_f_Wih"], inputs["gru1_f_Whh"], inputs["gru1_f_bih"], inputs["gru1_f_bhh"])
    p1b = (inputs["gru1_b_Wih"], inputs["gru1_b_Whh"], inputs["gru1_b_bih"], inputs["gru1_b_bhh"])
    params = [tuple(np.asarray(a, np.float32) for a in p) for p in (p0f, p0b, p1f, p1b)]
    x = np.asarray(inputs["input"], np.float32)
    eh = np.asarray(inputs["encoder_h"], np.float32)

    nc = _get_nc(S)
    in_maps = _prep_inputs(x, eh, params, S)
    res = run_bass_kernel_spmd(nc, in_maps, core_ids=list(range(NCORES)), trace=trace)
    outs = [res.results[r]["out"] for r in range(NCORES)]
    full = np.concatenate([o.transpose(1, 0, 2) for o in outs], axis=0)
    return full, res


def kernel(**inputs):
    full, _ = _run(inputs)
    return full, np.ascontiguousarray(full[:, -1, :])


# revision 18
# speedup vs baseline: 1.0875x; 1.0105x over previous
"""Bidirectional 2-layer GRU decoder on 8 TRN2 NeuronCores.

Strategy (pure SPMD, data-parallel over batch, no cross-core comms):
  - B=64 split 8 ways -> Bc=8 rows per core.
  - The 4 GRU cells (layer0 fwd, layer0 "bwd" (feature-flipped input),
    layer1 fwd, layer1 bwd) each own one PE column-group (32-partition
    quadrant): cell c -> PSUM partitions 32c..32c+7.
  - Input-side matmuls (gi = x@WihT + biasA) are precomputed in large
    batched matmuls: layer0 upfront into DRAM, layer1 in 16-step chunks
    from the h0^T history ring (so layer1 lags layer0 by LAG steps).
    Per step they are injected into each cell's PSUM quadrant with a
    K=128 identity matmul (a partition-aligning accumulate), then the
    recurrent gh matmuls accumulate on top.
  - Gates run as partition-parallel ACT/DVE/GpSimd ops spanning all
    quadrants; h'^T for the next step's stationary is rebuilt with tiny
    col-tiled selector matmuls into the spent r-gate PSUM bank.
  - Emission interleaves the 4 active cells (L0 @ step tau, L1 @ step
    tau-LAG) across the 4 PE column groups for concurrent streaming.
All matmul operands bf16 (fp32 PSUM accumulate), recurrent state fp32.
"""

import functools
import numpy as np
import ml_dtypes

import concourse.bass as bass
import concourse.mybir as mybir
import concourse.tile as tile
from concourse import bacc
from concourse.bass_utils import run_bass_kernel_spmd

try:
    import axon_prof
    axon_prof.install()
except Exception:
    pass

F32 = mybir.dt.float32
BF16 = mybir.dt.bfloat16
AF = mybir.ActivationFunctionType
OP = mybir.AluOpType

H = 512
B = 64
S_FULL = 512
NCORES = 8
BC = B // NCORES          # batch rows per core = 8
CH = 32                   # x-chunk size (phase-1 streaming)
CG = 16                   # gi1 chunk size in steps
LAG = 18                  # layer-1 step lag behind layer-0
PB = [0, 32, 64, 96]      # partition base per cell (0f, 0b, 1f, 1b)


def build_nc(S: int):
    ch = min(CH, S)
    nchunks = S // ch
    cg = min(CG, S)
    lag = LAG if S > CG else (cg + 2)
    nc = bacc.Bacc("TRN2")

    # ---- DRAM I/O ----
    ngrp = (S + 15) // 16
    xtf = nc.dram_tensor("xtf", [ngrp, 128, 512], BF16, kind="ExternalInput")
    xtb = nc.dram_tensor("xtb", [ngrp, 128, 512], BF16, kind="ExternalInput")
    wih = nc.dram_tensor("wih", [4, 128, 4 * 1536], BF16, kind="ExternalInput")
    whh = nc.dram_tensor("whh", [4, 128, 4 * 1536], BF16, kind="ExternalInput")
    biasn = nc.dram_tensor("biasn", [128, 512], BF16, kind="ExternalInput")
    biasa = nc.dram_tensor("biasa", [128, 4 * 1536], BF16, kind="ExternalInput")
    eye = nc.dram_tensor("eye", [128, 128], BF16, kind="ExternalInput")
    sel = nc.dram_tensor("sel", [128, 32], F32, kind="ExternalInput")
    h0i = nc.dram_tensor("h0i", [2, 128, 512], F32, kind="ExternalInput")
    hti = nc.dram_tensor("hti", [2, 128, 64], BF16, kind="ExternalInput")
    out = nc.dram_tensor("out", [S, BC, 2 * H], F32, kind="ExternalOutput")
    gi0d = nc.dram_tensor("gi0d", [(S + 15) // 16, 2, 128, 1536], BF16)  # internal

    with tile.TileContext(nc) as tc:
        with (
            tc.tile_pool(name="wpool", bufs=1) as wpool,
            tc.tile_pool(name="cpool", bufs=1) as cpool,
            tc.tile_pool(name="xpool", bufs=3) as xpool,
            tc.tile_pool(name="stpool", bufs=3) as stpool,
            tc.tile_pool(name="gpool", bufs=2) as gpool,
            tc.tile_pool(name="hpool", bufs=3) as hpool,
            tc.tile_pool(name="htpool", bufs=3) as htpool,
            tc.tile_pool(name="gi1pool", bufs=2) as gi1pool,
            tc.tile_pool(name="pA0", bufs=1, space="PSUM") as pA0,
            tc.tile_pool(name="pA1", bufs=1, space="PSUM") as pA1,
            tc.tile_pool(name="pN0", bufs=1, space="PSUM") as pN0,
            tc.tile_pool(name="pN1", bufs=1, space="PSUM") as pN1,
        ):
            # ---- constants / weights ----
            wih_sb = [wpool.tile([128, 4 * 1536], BF16, tag=f"wih{c}", name=f"wih{c}") for c in range(4)]
            whh_sb = [wpool.tile([128, 4 * 1536], BF16, tag=f"whh{c}", name=f"whh{c}") for c in range(4)]
            for c in range(4):
                nc.sync.dma_start(wih_sb[c][:], wih[c])
                nc.sync.dma_start(whh_sb[c][:], whh[c])
            biasn_sb = cpool.tile([128, 512], BF16, tag="biasn")
            biasa_sb = cpool.tile([128, 4 * 1536], BF16, tag="biasa")
            eye_sb = cpool.tile([128, 128], BF16, tag="eye")
            sel_sb = cpool.tile([128, 32], F32, tag="sel")
            nc.sync.dma_start(biasn_sb[:], biasn[:])
            nc.sync.dma_start(biasa_sb[:], biasa[:])
            nc.sync.dma_start(eye_sb[:], eye[:])
            nc.sync.dma_start(sel_sb[:], sel[:])

            psA = [pA0.tile([128, 1536], F32, tag="psA0", name="psA0"),
                   pA1.tile([128, 1536], F32, tag="psA1", name="psA1")]
            psN = [pN0.tile([128, 512], F32, tag="psN0", name="psN0"),
                   pN1.tile([128, 512], F32, tag="psN1", name="psN1")]

            # ---- phase 1: gi0 = x @ Wih0^T (+biasA) for both L0 cells ----
            for gi_ in range(ngrp):
                xf = xpool.tile([128, 512], BF16, tag="xsf", name=f"xf{gi_}")
                xb = xpool.tile([128, 512], BF16, tag="xsb", name=f"xb{gi_}")
                nc.sync.dma_start(xf[:], xtf[gi_])
                nc.sync.dma_start(xb[:], xtb[gi_])
                for cx, xs in ((0, xf), (1, xb)):
                    stg = stpool.tile([128, 1536], BF16, tag="stg",
                                      name=f"st{gi_}_{cx}")
                    for lo in (0, 512, 1024):
                        for k in range(4):
                            nc.tensor.matmul(
                                psA[0][0:128, 0:512],
                                xs[:, k * 128:k * 128 + 128],
                                wih_sb[cx][:, k * 1536 + lo:k * 1536 + lo + 512],
                                start=(k == 0), stop=(k == 3),
                                tile_position=(0, 0), skip_group_check=True)
                        nc.vector.tensor_tensor(
                            stg[:, lo:lo + 512],
                            psA[0][0:128, 0:512],
                            biasa_sb[:, cx * 1536 + lo:cx * 1536 + lo + 512],
                            OP.add)
                    nc.sync.dma_start(out=gi0d[gi_, cx, :, :], in_=stg[:, :])

            # zero psums (junk rows stay zero through the whole main loop)
            nc.vector.memset(psA[0][:], 0.0)
            nc.vector.memset(psA[1][:], 0.0)
            nc.vector.memset(psN[0][:], 0.0)
            nc.vector.memset(psN[1][:], 0.0)

            # ---- states ----
            h_prev = [hpool.tile([128, 512], F32, tag="hp0", name="hp0i"),
                      hpool.tile([128, 512], F32, tag="hp1", name="hp1i")]
            ht1_prev = htpool.tile([128, 64], BF16, tag="ht1", name="ht1i")
            ring = cpool.tile([128, 2048], BF16, tag="ring")   # h0^T history (2 halves)
            ht0_init = cpool.tile([128, 64], BF16, tag="ht0i")
            for L in range(2):
                nc.sync.dma_start(h_prev[L][:], h0i[L])
            nc.sync.dma_start(ht0_init[:], hti[0])
            nc.sync.dma_start(ht1_prev[:], hti[1])

            gi0_sb = {}   # chunk -> (tile_f, tile_b)
            gi1_sb = {}   # chunk -> (tile_f, tile_b)

            def gi0_chunk_load(C):
                tiles = []
                for cx in range(2):
                    g = gi1pool.tile([128, 1536], BF16, tag=f"gi0_{cx}",
                                     name=f"gi0c_{cx}_{C}")
                    nc.sync.dma_start(g[:], gi0d[C, cx])
                    tiles.append(g)
                gi0_sb[C] = tiles

            gi0_chunk_load(0)
            ring_v = ring.rearrange("p (hf k c d b) -> p hf k c d b",
                                    hf=2, k=4, c=2, d=16, b=8)

            SP = slice(0, 104)

            def ht0_slice(t, k, cx):
                if t < 0:
                    return ht0_init[:, k * 16 + 8 * cx:k * 16 + 8 * cx + 8]
                hf = (t // 16) % 2
                base = hf * 1024 + k * 256 + cx * 128 + (t % 16) * 8
                return ring[:, base:base + 8]

            def gi1_chunk_mm(C):
                """Batched gi1 for steps [C*cg, C*cg+cg) from the ring."""
                tiles = []
                ns = min(cg, S - C * cg)
                s0 = (C % 2) * 1024
                for cx in range(2):
                    c = 2 + cx
                    g = gi1pool.tile([128, 1536], BF16, tag=f"gi1_{cx}",
                                     name=f"gi1_{cx}_{C}")
                    if ns < cg or cg < 16:
                        nc.vector.memset(g[:, :], 0.0)
                    ps = psN[cx]
                    for lo in (0, 512, 1024):
                        for k in range(4):
                            lhsT = ring[:, s0 + k * 256 + cx * 128:
                                        s0 + k * 256 + cx * 128 + ns * 8]
                            nc.tensor.matmul(
                                ps[0:ns * 8, 0:512], lhsT,
                                wih_sb[c][:, k * 1536 + lo:k * 1536 + lo + 512],
                                start=(k == 0), stop=(k == 3),
                                tile_position=(0, 0), skip_group_check=True)
                        nc.vector.tensor_tensor(
                            g[0:ns * 8, lo:lo + 512], ps[0:ns * 8, 0:512],
                            biasa_sb[0:ns * 8, c * 1536 + lo:c * 1536 + lo + 512],
                            OP.add)
                    tiles.append(g)
                gi1_sb[C] = tiles

            def mm_slots(L, cx, t):
                """16 (out, lhsT, rhs, start, stop) slots for one cell-step."""
                c = 2 * L + cx
                pb = PB[c]
                A = psA[L]
                o = A[pb:pb + 8, :]
                oN = psN[L][pb:pb + 8, :]
                gsrc = (gi0_sb if L == 0 else gi1_sb)[t // cg][cx]
                esl = eye_sb[:, (t % cg) * 8:(t % cg) * 8 + 8]
                gl = [gsrc[:, lo:lo + 512] for lo in (0, 512, 1024)]
                hts = (lambda k: ht0_slice(t - 1, k, cx)) if L == 0 else \
                      (lambda k: ht1_prev[:, k * 16 + 8 * cx:k * 16 + 8 * cx + 8])
                slots = []
                # r bank: gi preload + gh-r
                slots.append((o[:, 0:512], esl, gl[0], True, False))
                for k in range(4):
                    slots.append((o[:, 0:512], hts(k),
                                  whh_sb[c][:, k * 1536:k * 1536 + 512],
                                  False, (k == 3)))
                # psN bank: bhn bias + gh-n
                slots.append((oN, eye_sb[:, 8 * c:8 * c + 8], biasn_sb[:], True, False))
                for k in range(4):
                    slots.append((oN, hts(k),
                                  whh_sb[c][:, k * 1536 + 1024:k * 1536 + 1536],
                                  False, (k == 3)))
                # n-gi bank: gi preload only
                slots.append((o[:, 1024:1536], esl, gl[2], True, True))
                # z bank: gi preload + gh-z
                slots.append((o[:, 512:1024], esl, gl[1], True, False))
                for k in range(4):
                    slots.append((o[:, 512:1024], hts(k),
                                  whh_sb[c][:, k * 1536 + 512:k * 1536 + 1024],
                                  False, (k == 3)))
                return pb, slots

            def gates(L, t):
                A = psA[L]
                pN_ = psN[L]
                sfx = f"{L}_{t}"
                r_t = gpool.tile([128, 512], BF16, tag=f"r{L}", name=f"r{sfx}")
                z_t = gpool.tile([128, 512], BF16, tag=f"z{L}", name=f"z{sfx}")
                oz = gpool.tile([128, 512], BF16, tag=f"oz{L}", name=f"oz{sfx}")
                zh = gpool.tile([128, 512], F32, tag=f"zh{L}", name=f"zh{sfx}")
                m1 = gpool.tile([128, 512], BF16, tag=f"m1{L}", name=f"m1{sfx}")
                a3 = gpool.tile([128, 512], BF16, tag=f"a3{L}", name=f"a3{sfx}")
                n_t = gpool.tile([128, 512], BF16, tag=f"n{L}", name=f"n{sfx}")
                m2 = gpool.tile([128, 512], BF16, tag=f"m2{L}", name=f"m2{sfx}")
                hp = hpool.tile([128, 512], F32, tag=f"hp{L}", name=f"hp{sfx}")
                nsb = gpool.tile([128, 512], BF16, tag=f"nsb{L}", name=f"nsb{sfx}")
                nc.scalar.activation(nsb[SP], pN_[SP, :], AF.Copy)
                nc.scalar.activation(r_t[SP], A[SP, 0:512], AF.Sigmoid)
                nc.vector.tensor_tensor(m1[SP], r_t[SP], nsb[SP], OP.mult)
                nc.vector.tensor_tensor(a3[SP], m1[SP], A[SP, 1024:1536], OP.add)
                nc.scalar.activation(z_t[SP], A[SP, 512:1024], AF.Sigmoid)
                # oz = 1 - z ; zh = z * h   (off critical path)
                nc.vector.tensor_scalar(oz[SP], z_t[SP], -1.0, 1.0,
                                        OP.mult, OP.add)
                nc.gpsimd.tensor_tensor(zh[SP], z_t[SP], h_prev[L][SP], OP.mult)
                nc.scalar.activation(n_t[SP], a3[SP], AF.Tanh)
                nc.vector.tensor_tensor(m2[SP], oz[SP], n_t[SP], OP.mult)
                nc.vector.tensor_tensor(hp[SP], m2[SP], zh[SP], OP.add)
                # h'^T selector matmuls into the spent r bank (cols 0:64)
                for k in range(4):
                    for g in range(4):
                        nc.tensor.matmul(
                            A[32 * g:32 * g + 32, 512 + 16 * k:512 + 16 * k + 16],
                            hp[0:104, 128 * k + 32 * g:128 * k + 32 * g + 32],
                            sel_sb[0:104, 16 * L:16 * L + 16],
                            start=True, stop=True, tile_position=(0, 32 * g),
                            skip_group_check=True)
                h_prev[L] = hp
                if L == 0:
                    hf = (t // 16) % 2
                    nc.vector.tensor_copy(
                        ring_v[:, hf, :, :, t % 16, :],
                        A[:, 512:576].rearrange("p (k c b) -> p k c b", k=4, c=2))
                    return hp, None
                nht = htpool.tile([128, 64], BF16, tag="ht1", name=f"ht1_{t}")
                nc.vector.tensor_copy(nht[:], A[:, 512:576])
                return hp, nht

            # ---- main loop ----
            for tau in range(S + lag):
                if tau % cg == 0 and tau >= cg:
                    C = (tau - cg) // cg
                    if C * cg < S:
                        gi1_chunk_mm(C)
                if tau % cg == 0 and tau + cg < S:
                    gi0_chunk_load((tau + cg) // cg)

                work = []
                if tau < S:
                    work.append(mm_slots(0, 0, tau))
                    work.append(mm_slots(0, 1, tau))
                t1 = tau - lag
                if 0 <= t1 < S:
                    work.append(mm_slots(1, 0, t1))
                    work.append(mm_slots(1, 1, t1))
                for i in range(16):
                    for pb, slots in work:
                        o_ap, lhsT, rhs, st_, sp_ = slots[i]
                        nc.tensor.matmul(o_ap, lhsT, rhs, start=st_, stop=sp_,
                                         tile_position=(0, pb),
                                         skip_group_check=True)
                if tau < S:
                    gates(0, tau)
                if 0 <= t1 < S:
                    hp1, nht = gates(1, t1)
                    ht1_prev = nht
                    nc.sync.dma_start(out[t1, :, 0:512], hp1[64:72, :])
                    nc.sync.dma_start(out[t1, :, 512:1024], hp1[96:104, :])

    nc.compile()
    return nc


# ---------------- host-side data prep ----------------

def _gru_splits(Wih, Whh, bih, bhh):
    wihT = np.ascontiguousarray(Wih.T)
    whhT = np.ascontiguousarray(Whh.T)
    wih_sw = wihT.reshape(4, 128, 1536).transpose(1, 0, 2).reshape(128, 4 * 1536)
    whh_sw = whhT.reshape(4, 128, 1536).transpose(1, 0, 2).reshape(128, 4 * 1536)
    biasA = np.concatenate([bih[0:512] + bhh[0:512],
                            bih[512:1024] + bhh[512:1024],
                            bih[1024:1536]])
    biasB = bhh[1024:1536]
    return wih_sw, whh_sw, biasA, biasB


@functools.lru_cache(maxsize=2)
def _get_nc(S):
    return build_nc(S)


def _prep_inputs(input, encoder_h, params, S):
    ngrp = (S + 15) // 16
    bf = ml_dtypes.bfloat16

    wih_all = np.zeros((4, 128, 4 * 1536), np.float32)
    whh_all = np.zeros((4, 128, 4 * 1536), np.float32)
    biasn = np.zeros((128, 512), np.float32)
    biasa = np.zeros((128, 4 * 1536), np.float32)
    for c in range(4):
        wih_sw, whh_sw, biasA, biasB = _gru_splits(*params[c])
        wih_all[c] = wih_sw
        whh_all[c] = whh_sw
        biasa[:, c * 1536:(c + 1) * 1536] = biasA[None, :]
        biasn[8 * c:8 * c + 8, :] = biasB[None, :]
    eye = np.eye(128, dtype=np.float32)
    sel = np.zeros((128, 32), np.float32)
    for L in range(2):
        for cx in range(2):
            for j in range(8):
                sel[64 * L + 32 * cx + j, 16 * L + 8 * cx + j] = 1.0

    shared = {
        "wih": wih_all.astype(bf),
        "whh": whh_all.astype(bf),
        "biasn": biasn.astype(bf),
        "biasa": biasa.astype(bf),
        "eye": eye.astype(bf),
        "sel": sel,
    }

    per_core = []
    for r in range(NCORES):
        bs = slice(r * BC, (r + 1) * BC)
        xc = input[bs, :S]

        def swz(xarr):
            xp = np.zeros((BC, ngrp * 16, 512), np.float32)
            xp[:, :S] = xarr
            # [b, g, dt, k, p] -> [g, p, k, dt, b]
            xt = xp.reshape(BC, ngrp, 16, 4, 128).transpose(1, 4, 3, 2, 0)
            return np.ascontiguousarray(xt).reshape(ngrp, 128, 512).astype(bf)
        xtf = swz(xc)
        xtb = swz(xc[:, :, ::-1])
        hf = encoder_h[bs, 0:512]
        hb = encoder_h[bs, 512:1024]
        h0i = np.zeros((2, 128, 512), np.float32)
        hti = np.zeros((2, 128, 64), np.float32)
        for L in range(2):
            h0i[L, 0 + 64 * L:8 + 64 * L] = hf
            h0i[L, 32 + 64 * L:40 + 64 * L] = hb
            for k in range(4):
                hti[L, :, 16 * k + 0:16 * k + 8] = hf.T[128 * k:128 * k + 128]
                hti[L, :, 16 * k + 8:16 * k + 16] = hb.T[128 * k:128 * k + 128]
        per_core.append(dict(shared, xtf=xtf, xtb=xtb,
                             h0i=h0i, hti=hti.astype(bf)))
    return per_core


def _run(inputs, S=S_FULL, trace=False):
    p0f = (inputs["gru0_f_Wih"], inputs["gru0_f_Whh"], inputs["gru0_f_bih"], inputs["gru0_f_bhh"])
    p0b = (inputs["gru0_b_Wih"], inputs["gru0_b_Whh"], inputs["gru0_b_bih"], inputs["gru0_b_bhh"])
    p1f = (inputs["gru1_f_Wih"], inputs["gru1_f_Whh"], inputs["gru1_f_bih"], inputs["gru1_f_bhh"])
    p1b = (inputs["gru1_b_Wih"], inputs["gru1_b_Whh"], inputs["gru1_b_bih"], inputs["gru1_b_bhh"])
    params = [tuple(np.asarray(a, np.float32) for a in p) for p in (p0f, p0b, p1f, p1b)]
    x = np.asarray(inputs["input"], np.float32)
    eh = np.asarray(inputs["encoder_h"], np.float32)

    nc = _get_nc(S)
    in_maps = _prep_inputs(x, eh, params, S)
    res = run_bass_kernel_spmd(nc, in_maps, core_ids=list(range(NCORES)), trace=trace)
    outs = [res.results[r]["out"] for r in range(NCORES)]
    full = np.concatenate([o.transpose(1, 0, 2) for o in outs], axis=0)
    return full, res


def kernel(**inputs):
    full, _ = _run(inputs)
    return full, np.ascontiguousarray(full[:, -1, :])
